# revision 29
# baseline (speedup 1.0000x reference)
"""Trainium2 Bass kernel for nn_MultiHeadAttention (B=2, T=2048, C=1024, H=16).

Sharding: 8 NeuronCores = 2 batch groups x 4 tensor-parallel cores.
Core c handles batch b = c // 4 and heads h0..h0+3, h0 = (c % 4) * 4.
Each core computes the qkv projection for its head slice, causal attention
for its 4 heads, and a partial output projection (rows of W_out for its
heads). Host glue: slice weights per core, sum the 4 TP partials per batch,
add b_out.

Device dataflow (matmuls in bf16, fp32 accumulation):
  A: x [T,C] f32 -DMA-> SBUF -PE transpose(f32)-> PSUM -ACT cast-> xT bf16
  B: qkT [512,T] = W_qk^T @ x^T (+bias)         (q,k head-major)
  C: V   [T,4,65] = x @ W_v (+bias, ones col)   (t-natural, 65 cols/head)
  D: per head pair (PE row-groups 0-63/64-127 run concurrently):
     S^T[j,i] = K^T_j^T Q^T_i ; P = exp(S^T/8) * causal_mask
     O^T|colsum = (V|1)^T @ P  (PSUM accum over j)
     O^T *= 1/colsum (reciprocal on DVE or ACT, broadcast via DRAM DMA)
  E: y_partial [T,C] = (O^T)^T @ W_out_rows  (interleaved per query half)

A,B,C are emitted interleaved per 512-row group of x so the PE starts the
projection matmuls as soon as the first quarter of x^T is available.
"""

import sys

sys.path.insert(0, "/opt/trn_rl_repo")

import numpy as np
import ml_dtypes

import concourse.bass as bass
import concourse.mybir as mybir
from concourse.tile import TileContext
from concourse.bass_utils import run_bass_kernel_spmd
from concourse.masks import make_identity

T = 2048
C = 1024
H = 16
D = 64
NCORE = 8
TPG = 4          # tensor-parallel group size (cores per batch)
HC = H // TPG    # heads per core
CL = HC * D      # local c dim (256)
F32 = mybir.dt.float32
BF16 = mybir.dt.bfloat16
AF = mybir.ActivationFunctionType

NT = T // 128    # 16 t-tiles
NCB = C // 128   # 8 c-tiles
NIC = T // 512   # 4 512-query chunks


def _build_program():
    nc = bass.Bass("TRN2", target_bir_lowering=False, debug=False)

    x = nc.declare_dram_parameter("x", [T, C], F32, isOutput=False)
    wqk = nc.declare_dram_parameter("wqk", [C, 2 * CL], F32, isOutput=False)
    bqk = nc.declare_dram_parameter("bqk", [2 * CL], F32, isOutput=False)
    wv = nc.declare_dram_parameter("wv", [C, CL], F32, isOutput=False)
    bv = nc.declare_dram_parameter("bv", [CL], F32, isOutput=False)
    wo = nc.declare_dram_parameter("wo", [CL, C], F32, isOutput=False)
    trimask = nc.declare_dram_parameter("trimask", [128, 128], BF16, isOutput=False)
    y = nc.declare_dram_parameter("y", [T, C], F32, isOutput=True)

    with TileContext(nc) as tc:
        with (
            tc.tile_pool(name="singles", bufs=1) as singles,
            tc.tile_pool(name="xstage", bufs=4) as xstage,
            tc.tile_pool(name="ptp", bufs=6) as ptp,
            tc.tile_pool(name="small", bufs=6) as small,
            tc.tile_pool(name="yout", bufs=6) as yout,
            tc.tile_pool(name="dram", bufs=1, space="DRAM") as dram,
            tc.tile_pool(name="psum", bufs=2, space="PSUM") as pp,
        ):
            # ---- persistent SBUF tensors ----
            xT = singles.tile([128, NCB, T], BF16)        # x^T, c on partitions
            wqk_sb = singles.tile([128, NCB, 2 * CL], BF16)
            wv_sb = singles.tile([128, NCB, CL], BF16)
            wo_sb = singles.tile([128, 2, C], BF16)
            qkT = singles.tile([128, 4, T], BF16)         # [q01,q23,k01,k23]
            qkD = singles.tile([128, 4, T], BF16)         # qkT, row halves swapped
            ones_sb = singles.tile([1, 64], F32)
            v_sb = singles.tile([128, NT, HC, D + 1], BF16)
            ot_sb = singles.tile([128, 2, T], BF16)       # O^T, c_local on part
            bqk_sb = singles.tile([128, 4], F32)
            bv_sb = singles.tile([128, CL], F32)
            mask_sb = singles.tile([128, 128], BF16)
            ident = singles.tile([128, 128], F32)

            # ---- constants / weights ----
            make_identity(nc, ident)
            nc.vector.memset(ones_sb, 1.0)
            nc.sync.dma_start(out=mask_sb, in_=trimask[:, :])
            for m in range(4):
                nc.sync.dma_start(
                    out=bqk_sb[:, m : m + 1], in_=bqk[m * 128 : (m + 1) * 128, None]
                )
            nc.gpsimd.dma_start(out=bv_sb, in_=bv[None, :].to_broadcast((128, CL)))
            for kc in range(NCB):
                nc.gpsimd.dma_start(
                    out=wqk_sb[:, kc, :], in_=wqk[kc * 128 : (kc + 1) * 128, :]
                )
                nc.gpsimd.dma_start(
                    out=wv_sb[:, kc, :], in_=wv[kc * 128 : (kc + 1) * 128, :]
                )
            for kc in range(2):
                nc.gpsimd.dma_start(
                    out=wo_sb[:, kc, :], in_=wo[kc * 128 : (kc + 1) * 128, :]
                )

            # ---- phases A+B+C interleaved per 512-row group of x ----
            for grp in range(NIC):
                # A: load 4 x-tiles, PE-transpose, ACT-copy into xT (bf16)
                for tt in range(4 * grp, 4 * grp + 4):
                    xf = xstage.tile([128, C], F32, tag="xf")
                    nc.sync.dma_start(out=xf, in_=x[tt * 128 : (tt + 1) * 128, :])
                    for half in range(2):
                        tr = pp.tile(
                            [128, 512], F32, tag="bc", name=f"tr_{tt}_{half}"
                        )
                        for cc in range(4):
                            cb = half * 4 + cc
                            nc.tensor.transpose(
                                tr[:, cc * 128 : (cc + 1) * 128],
                                xf[:, cb * 128 : (cb + 1) * 128],
                                ident,
                            )
                        nc.scalar.copy(
                            out=xT[:, half * 4 : half * 4 + 4, tt * 128 : (tt + 1) * 128],
                            in_=tr.rearrange("p (cb t) -> p cb t", cb=4),
                        )
                # B: q,k columns for this 512-query chunk
                for m in range(4):
                    ps = pp.tile([128, 512], F32, tag="bc", name=f"qk_{grp}_{m}")
                    for kc in range(NCB):
                        nc.tensor.matmul(
                            ps[:, 0:512],
                            lhsT=wqk_sb[:, kc, m * 128 : (m + 1) * 128],
                            rhs=xT[:, kc, grp * 512 : (grp + 1) * 512],
                            start=(kc == 0),
                            stop=(kc == NCB - 1),
                        )
                    nc.vector.tensor_scalar_add(
                        out=qkT[:, m, grp * 512 : (grp + 1) * 512],
                        in0=ps[:, 0:512],
                        scalar1=bqk_sb[:, m : m + 1],
                    )
                    nc.gpsimd.dma_start(
                        out=qkD[64:128, m, grp * 512 : (grp + 1) * 512],
                        in_=qkT[0:64, m, grp * 512 : (grp + 1) * 512],
                    )
                    nc.gpsimd.dma_start(
                        out=qkD[0:64, m, grp * 512 : (grp + 1) * 512],
                        in_=qkT[64:128, m, grp * 512 : (grp + 1) * 512],
                    )
                # C: V rows for these 4 t-tiles
                for tt in range(4 * grp, 4 * grp + 4):
                    ps = pp.tile([128, 512], F32, tag="bc", name=f"v_{tt}")
                    for kc in range(NCB):
                        nc.tensor.matmul(
                            ps[:, 0:CL],
                            lhsT=xT[:, kc, tt * 128 : (tt + 1) * 128],
                            rhs=wv_sb[:, kc, :],
                            start=(kc == 0),
                            stop=(kc == NCB - 1),
                        )
                    nc.vector.tensor_tensor(
                        out=v_sb[:, tt, :, 0:D],
                        in0=ps[:, 0:CL].rearrange("p (h d) -> p h d", h=HC),
                        in1=bv_sb.rearrange("p (h d) -> p h d", h=HC),
                        op=mybir.AluOpType.add,
                    )
                    nc.vector.memset(v_sb[:, tt, :, D : D + 1], 1.0)

            # ---- phase D: attention per query half, per head; E interleaved ----
            for ic2 in range(2):
                c0 = ic2 * 1024
                n_jt = 8 * (ic2 + 1)
                for h in range(HC):
                    pb = (h % 2) * 64
                    po = 64 - pb  # swapped-half offset in qkD
                    ots = [
                        pp.tile([65, 512], F32, tag="ot", name=f"ot_{ic2}_{h}_{i}")
                        for i in range(2)
                    ]
                    for jt0 in range(0, n_jt, 2):
                        jts = [jt for jt in (jt0, jt0 + 1) if jt < n_jt]
                        sts = {}
                        pts = {}
                        # S^T matmuls for the jt pair: even jt uses qkT rows,
                        # odd jt the swapped copy -> disjoint PE row groups,
                        # adjacent in the PE stream, run concurrently.
                        for jt in jts:
                            off = max(0, jt * 128 - c0)
                            if jt % 2 == 0:
                                qt = qkT[pb : pb + 64, h // 2, :]
                                kt = qkT[pb : pb + 64, 2 + h // 2, :]
                            else:
                                qt = qkD[po : po + 64, h // 2, :]
                                kt = qkD[po : po + 64, 2 + h // 2, :]
                            st = pp.tile(
                                [128, 1024], F32, tag="st",
                                name=f"st_{ic2}_{h}_{jt}",
                            )
                            sts[jt] = st
                            for sc in range(2):
                                lo = sc * 512
                                if lo + 512 <= off:
                                    continue
                                nc.tensor.matmul(
                                    st[:, lo : lo + 512],
                                    lhsT=kt[:, jt * 128 : (jt + 1) * 128],
                                    rhs=qt[:, c0 + lo : c0 + lo + 512],
                                    start=True,
                                    stop=True,
                                )
                        for jt in jts:
                            off = max(0, jt * 128 - c0)
                            pt = ptp.tile(
                                [128, 1024], BF16, tag="pt",
                                name=f"pt_{ic2}_{h}_{jt}",
                            )
                            pts[jt] = pt
                            nc.scalar.activation(
                                out=pt[:, off:1024],
                                in_=sts[jt][:, off:1024],
                                func=AF.Exp,
                                scale=0.125,
                            )
                            if jt * 128 >= c0:
                                nc.vector.tensor_mul(
                                    pt[:, off : off + 128],
                                    pt[:, off : off + 128],
                                    mask_sb,
                                )
                        for jt in jts:
                            off = max(0, jt * 128 - c0)
                            for sc in range(2):
                                lo = sc * 512
                                a = max(off, lo)
                                if a >= lo + 512:
                                    continue
                                last_jt = (8 * ic2 + 4 * sc + 4) - 1
                                nc.tensor.matmul(
                                    ots[sc][:, a - lo : 512],
                                    lhsT=v_sb[:, jt, h, :],
                                    rhs=pts[jt][:, a : lo + 512],
                                    start=(jt == 0),
                                    stop=(jt == last_jt),
                                )
                    for sc in range(2):
                        rec = small.tile([1, 512], F32, tag="rec")
                        lnc = small.tile([1, 512], F32, tag="lnc")
                        nc.scalar.activation(
                            out=lnc, in_=ots[sc][64:65, :], func=AF.Ln
                        )
                        nc.scalar.activation(
                            out=rec, in_=lnc, func=AF.Exp, scale=-1.0
                        )
                        bc_ps = pp.tile(
                            [128, 512], F32, tag="bc", name=f"bc_{ic2}_{h}_{sc}"
                        )
                        nc.tensor.matmul(
                            bc_ps[0:64, :], lhsT=ones_sb, rhs=rec,
                            start=True, stop=True,
                        )
                        bc_sb = small.tile([64, 512], F32, tag="bcs")
                        nc.vector.tensor_copy(bc_sb, bc_ps[0:64, :])
                        nc.vector.tensor_mul(
                            ot_sb[
                                pb : pb + 64,
                                h // 2,
                                c0 + sc * 512 : c0 + (sc + 1) * 512,
                            ],
                            ots[sc][0:64, :],
                            bc_sb,
                        )

                # E: out-projection for this query half
                for tt in range(8 * ic2, 8 * ic2 + 8):
                    for nch in range(2):
                        ps = pp.tile(
                            [128, 512], F32, tag="bc", name=f"y_{tt}_{nch}"
                        )
                        for kc in range(2):
                            nc.tensor.matmul(
                                ps[:, 0:512],
                                lhsT=ot_sb[:, kc, tt * 128 : (tt + 1) * 128],
                                rhs=wo_sb[:, kc, nch * 512 : (nch + 1) * 512],
                                start=(kc == 0),
                                stop=(kc == 1),
                            )
                        ys = yout.tile([128, 512], F32)
                        if (tt + nch) % 2 == 0:
                            nc.vector.tensor_copy(ys, ps[:, 0:512])
                        else:
                            nc.scalar.copy(out=ys, in_=ps[:, 0:512])
                        nc.sync.dma_start(
                            out=y[
                                tt * 128 : (tt + 1) * 128,
                                nch * 512 : (nch + 1) * 512,
                            ],
                            in_=ys,
                        )

    _split_multi_waits(nc)
    return nc


_WAIT_CTR = [0]


def _split_multi_waits(nc, max_waits=1):
    """This container's walrus accepts only ONE sem wait per instruction.
    Hoist extra waits onto standalone EventSemaphore insts just before."""
    for f in nc.m.functions:
        for bb in f.blocks:
            insts = list(bb.instructions)
            out = []
            changed = False
            for inst in insts:
                si = inst.sync_info
                if si is not None and len(si.on_wait) > max_waits:
                    waits = list(si.on_wait)
                    keep, extra = waits[-max_waits:], waits[:-max_waits]
                    for w in extra:
                        _WAIT_CTR[0] += 1
                        out.append(
                            mybir.InstEventSemaphore(
                                name=f"xw-{_WAIT_CTR[0]}",
                                engine=inst.engine,
                                ins=[],
                                outs=[],
                                sync_info=mybir.SyncInfo(on_wait=[w], on_update=[]),
                            )
                        )
                    inst.sync_info = mybir.SyncInfo(
                        on_wait=keep, on_update=list(si.on_update)
                    )
                    changed = True
                out.append(inst)
            if changed:
                bb.instructions = out


_PROGRAM = None


def _get_program():
    global _PROGRAM
    if _PROGRAM is None:
        _PROGRAM = _build_program()
    return _PROGRAM


def _make_in_maps(x, W_attn, b_attn, W_out, b_out):
    bf16 = ml_dtypes.bfloat16
    tri = np.triu(np.ones((128, 128), dtype=bf16))  # mask[j, i] = j <= i
    in_maps = []
    for core in range(NCORE):
        b = core // TPG
        h0 = (core % TPG) * HC
        qcols = slice(h0 * D, (h0 + HC) * D)
        kcols = slice(C + h0 * D, C + (h0 + HC) * D)
        vcols = slice(2 * C + h0 * D, 2 * C + (h0 + HC) * D)
        in_maps.append(
            {
                "x": np.ascontiguousarray(x[b]),
                "wqk": np.ascontiguousarray(
                    np.concatenate([W_attn[:, qcols], W_attn[:, kcols]], axis=1)
                ),
                "bqk": np.ascontiguousarray(
                    np.concatenate([b_attn[qcols], b_attn[kcols]])
                ),
                "wv": np.ascontiguousarray(W_attn[:, vcols]),
                "bv": np.ascontiguousarray(b_attn[vcols]),
                "wo": np.ascontiguousarray(W_out[h0 * D : (h0 + HC) * D, :]),
                "trimask": tri,
            }
        )
    return in_maps


def _run(x, W_attn, b_attn, W_out, b_out, trace=False):
    nc = _get_program()
    in_maps = _make_in_maps(x, W_attn, b_attn, W_out, b_out)
    res = run_bass_kernel_spmd(nc, in_maps, list(range(NCORE)), trace=trace)
    parts = [res.results[i]["y"].astype(np.float32) for i in range(NCORE)]
    out = np.stack(
        [
            parts[0] + parts[1] + parts[2] + parts[3],
            parts[4] + parts[5] + parts[6] + parts[7],
        ]
    )
    out += b_out.astype(np.float32)
    return out, res


def kernel(x, W_attn, b_attn, W_out, b_out):
    out, _ = _run(
        np.asarray(x), np.asarray(W_attn), np.asarray(b_attn),
        np.asarray(W_out), np.asarray(b_out),
    )
    return out
